# revision 34
# baseline (speedup 1.0000x reference)
"""Distributed exact cosine top-k retrieval (MemoryBank) on 8 trn2 NeuronCores.

Strategy (v4: split-PSUM cross-chunk fused drain+fold, fp8 keys):
  - memory_keys sharded row-wise across 8 cores; queries replicated.
  - Host prep: L2-normalize keys/queries in fp64; queries bf16, keys fp8
    e4m3 scaled x64 (all entries in the normal range; the candidate screen
    is scale-invariant and the host rescore is exact, so fp8 only adds
    ~0.003 screen noise vs a 0.049 validated margin).  Pre-transposed to
    [D, n]; key shard is 8 MB/core and stays RESIDENT in SBUF.
  - Per core, per query block (8 x 128 queries), stream 31 chunks of 2048
    keys through PSUM.  Each chunk uses TWO separate PSUM tiles (pshi =
    keys [1024:2048], pslo = keys [0:1024], 2 banks each, double
    buffered = all 8 banks).  The split matters: the Tile tracker
    serializes two engine readers of the SAME PSUM tile, which would kill
    ACT/DVE overlap.
  - Drain (the wall): sims leave PSUM as fp32 (TRN2 matmul cannot write
    16-bit PSUM) at 1 elem/cyc per engine port.  Per chunk, concurrently:
      ACT: copy pshi(c) -> SBUF bf16 "schi" (~1.11us saturated cadence)
      DVE: tensor_tensor MAX(pslo(c), schi(c-1)) -> stage bf16 (~1.13us)
    i.e. the pair-max fold is FUSED into the DVE's drain pass, pairing
    across adjacent chunks so ACT(c) and DVE(c) have no intra-chunk
    dependency.  Steady state is DVE-paced at ~1134ns/chunk with both
    drain engines >95% busy -- the PSUM-read duopoly floor.  (Engine
    rebalancing was measured: shifting whole chunks to ACT-only W=1
    shipping is a net loss; ACT saturates at ~1.12us/copy.)
  - PE: 4 bf16xfp8 matmuls (N=512, one PSUM bank each) per chunk at
    ~215ns issue gap; queries first in the DMA queue so compute starts at
    ~11us, not after the full key load.
  - Output: per (query block, core) 32 slots x 1024 bf16 pair-maxes
    (slot 0 pairs with a -3 junk partner; slot 31 = ACT's copy of the
    last chunk's hi half, written straight into the stage tile), shipped
    in 1 MiB staged DMAs, 65 MB/core total, overlapped.  qb0 is DMA-bound
    (key load + outputs share ~390 GB/s), so the first output stages are
    held back via tile_wait_until until the key load drains, with stage
    bufs=6 absorbing the staged folds in SBUF meanwhile.
  - Host: for each (query, core) take the top-NSEL pair-max slots, expand
    to member key ids, rescore candidates exactly in fp64 (normalized dot
    = cosine), global top-8 with the reference tie-break (desc sim, asc
    index), assemble output rows from memory_values.

Pair membership: slot (c, j), c in 0..30: members {c*2048+j} for c=0 else
{c*2048+j, (c-1)*2048+1024+j}; slot (31, j): {30*2048+1024+j}.

Selection depth NSEL=32 pairs/core, validated offline on the real dataset
(validate_screen.py): zero misses, worst margin 0.0488 for bf16 keys and
3.125/64 = 0.0488 for fp8 keys -- ~15 sigma above the sim noise.

History (HW-measured, rel err 0.0 throughout):
  1511555 ns  naive predecessor
   472542 ns  staged baseline (fp32 PSUM drained by match_replace/ACT at
              1x, fold as a separate DVE pass; DVE 85% busy on overhead)
   347290 ns  split-PSUM tiles + fused cross-chunk drain+fold
   343868 ns  + queries loaded before keys
   307917 ns  + fp8 keys, ACT-written edge slot, stage bufs 4, QSTEP 4
   303252 ns  + qb0 output-DMA deferral (tile_wait_until), stage bufs 6,
              split final stage DMA
"""

import numpy as np

import concourse.bacc as bacc
import concourse.bass as bass
import concourse.mybir as mybir
from concourse import tile
from concourse.bass_utils import run_bass_kernel_spmd

# problem sizes (hardcoded per contract)
B = 1024
N = 500000
D = 128
TOPK = 8
NCORES = 8
NLOC = N // NCORES  # 62500
CHUNK = 2048
NCHUNKS = (NLOC + CHUNK - 1) // CHUNK  # 31
NPAD = NCHUNKS * CHUNK  # 63488
P = 128
NQB = B // P  # 8 query blocks
GPC = CHUNK // 2  # 1024 pair-maxes per output slot
NSLOT = NCHUNKS + 1  # 31 chunk slots + 1 edge slot (last chunk's hi half)
OUTW = NSLOT * GPC  # 32768 pair-maxes per (query, core)

# host selection depth: top-NSEL pairs per (query, core) -> 2*NSEL candidate
# rows rescored exactly.
NSEL = 32

QSTEP = 4  # slots per staged out-DMA (4 * 1024 * 2B * 128 = 1 MiB)

_dt = mybir.dt


def build_kernel():
    """Build the per-core Bass program (SPMD: same program, different data)."""
    nc = bacc.Bacc(None, target_bir_lowering=False, debug=False)
    dt = _dt

    kT = nc.dram_tensor("kT", [P, NPAD], dt.float8e4, kind="ExternalInput")
    qT = nc.dram_tensor("qT", [P, B], dt.bfloat16, kind="ExternalInput")
    pm = nc.dram_tensor("pm", [B, OUTW], dt.bfloat16, kind="ExternalOutput")

    with tile.TileContext(nc) as tc:
        with (
            tc.tile_pool(name="kres", bufs=1) as kres,
            tc.tile_pool(name="qpool", bufs=1) as qpool,
            tc.tile_pool(name="scr", bufs=3) as scr,
            tc.tile_pool(name="stage", bufs=6) as stage,
            tc.tile_pool(name="psum", bufs=2, space="PSUM") as psum,
        ):
            # queries first (tiny, needed by every matmul; qb0's slice on its
            # own DMA so the first matmul can start earliest), then the
            # resident key shard chunk-by-chunk so qb0 compute starts as soon
            # as chunk 0 lands
            qt = qpool.tile([P, B], dt.bfloat16)
            nc.sync.dma_start(qt[:, 0:P], qT.ap()[:, 0:P])
            nc.sync.dma_start(qt[:, P:B], qT.ap()[:, P:B])
            kt = kres.tile([P, NPAD], dt.float8e4)
            for ch in range(NCHUNKS):
                nc.sync.dma_start(
                    kt[:, ch * CHUNK : (ch + 1) * CHUNK],
                    kT.ap()[:, ch * CHUNK : (ch + 1) * CHUNK],
                )
            # pair partner for slot 0 (never wins: all sims > -3)
            junk = qpool.tile([P, GPC], dt.bfloat16)
            nc.vector.memset(junk[:], -3.0)

            for qb in range(NQB):
                schi_prev = junk
                for s0 in range(0, NSLOT, QSTEP):
                    nslot = min(QSTEP, NSLOT - s0)
                    st = stage.tile([P, QSTEP * GPC], dt.bfloat16, tag="st")
                    for si in range(nslot):
                        s = s0 + si
                        if s < NCHUNKS:
                            c = s
                            # split lo/hi PSUM tiles so ACT (reads pshi) and
                            # DVE (reads pslo) never touch the same PSUM tile
                            # -- the Tile tracker serializes same-tile readers
                            pshi = psum.tile([P, GPC], dt.float32, tag="pshi")
                            pslo = psum.tile([P, GPC], dt.float32, tag="pslo")
                            for j in range(2):
                                nc.tensor.matmul(
                                    out=pshi[:, j * 512 : (j + 1) * 512],
                                    lhsT=qt[:, qb * P : (qb + 1) * P],
                                    rhs=kt[
                                        :,
                                        c * CHUNK + GPC + j * 512 : c * CHUNK
                                        + GPC
                                        + (j + 1) * 512,
                                    ],
                                    start=True,
                                    stop=True,
                                )
                            for j in range(2):
                                nc.tensor.matmul(
                                    out=pslo[:, j * 512 : (j + 1) * 512],
                                    lhsT=qt[:, qb * P : (qb + 1) * P],
                                    rhs=kt[
                                        :,
                                        c * CHUNK + j * 512 : c * CHUNK + (j + 1) * 512,
                                    ],
                                    start=True,
                                    stop=True,
                                )
                            # ACT: drain hi half -> SBUF bf16 (next chunk's
                            # pair partner).  For the last chunk the copy
                            # goes straight into the edge stage slot.
                            if c < NCHUNKS - 1:
                                schi_t = scr.tile(
                                    [P, GPC], dt.bfloat16, tag="sc", name="schi"
                                )
                                schi = schi_t[:]
                            else:
                                schi = st[:, (si + 1) * GPC : (si + 2) * GPC]
                            nc.scalar.copy(schi, pshi[:])
                            # DVE: fused drain+fold of lo half against the
                            # previous chunk's hi half
                            nc.vector.tensor_tensor(
                                out=st[:, si * GPC : (si + 1) * GPC],
                                in0=pslo[:],
                                in1=schi_prev,
                                op=mybir.AluOpType.max,
                            )
                            schi_prev = schi
                    if qb == 0 and s0 <= 12:
                        # qb0 is DMA-bound (key load + output stages share
                        # ~390 GB/s); hold the first output stages back so
                        # the resident-key load finishes unimpeded
                        with tc.tile_wait_until(0.028 + s0 * 0.0004):
                            nc.sync.dma_start(
                                pm.ap()[
                                    qb * P : (qb + 1) * P,
                                    s0 * GPC : (s0 + nslot) * GPC,
                                ],
                                st[:, : nslot * GPC],
                            )
                    elif qb == NQB - 1 and s0 == NSLOT - QSTEP:
                        # final stage: ship in two halves so the kernel tail
                        # only waits on a 0.5 MiB transfer
                        half = nslot // 2
                        nc.sync.dma_start(
                            pm.ap()[
                                qb * P : (qb + 1) * P,
                                s0 * GPC : (s0 + half) * GPC,
                            ],
                            st[:, : half * GPC],
                        )
                        nc.sync.dma_start(
                            pm.ap()[
                                qb * P : (qb + 1) * P,
                                (s0 + half) * GPC : (s0 + nslot) * GPC,
                            ],
                            st[:, half * GPC : nslot * GPC],
                        )
                    else:
                        nc.sync.dma_start(
                            pm.ap()[
                                qb * P : (qb + 1) * P,
                                s0 * GPC : (s0 + nslot) * GPC,
                            ],
                            st[:, : nslot * GPC],
                        )

    nc.compile()
    return nc


_NC_CACHE = {}

# test-harness knobs (the grading harness leaves these at defaults)
TRACE = False
LAST_EXEC_NS = None
LAST_RESULTS = None


def _get_nc(key):
    if key not in _NC_CACHE:
        _NC_CACHE[key] = build_kernel()
    return _NC_CACHE[key]


def _install_trace_shim():
    """Register the missing antenv.axon_hooks NTFF profile hook (dev only)."""
    import sys
    import types

    if "antenv.axon_hooks" in sys.modules:
        return
    from trn_agent_boot.trn_boot import _ntff_profile_via_ctypes

    hooks = types.ModuleType("antenv.axon_hooks")
    impl = _ntff_profile_via_ctypes("/opt/axon/libaxon_pjrt.so")
    hooks.get_axon_ntff_profile_hook = lambda: impl
    hooks.set_axon_ntff_profile_hook = lambda h: None
    sys.modules["antenv.axon_hooks"] = hooks

    import concourse.bass_utils as bu

    bu.upload_artifacts = lambda tmpdir: f"local:{tmpdir}"


def _member_table():
    """Map shipped column t in [0, OUTW) to its <=2 member key rows
    (shard-local, may exceed NLOC for zero-padded tail; -1 = no member)."""
    t_all = np.arange(OUTW, dtype=np.int64)
    s_all, j_all = np.divmod(t_all, GPC)
    m0 = np.where(s_all < NCHUNKS, s_all * CHUNK + j_all, (NCHUNKS - 1) * CHUNK + GPC + j_all)
    m1 = np.where(
        (s_all >= 1) & (s_all < NCHUNKS), (s_all - 1) * CHUNK + GPC + j_all, -1
    )
    return np.stack([m0, m1], axis=1)  # [OUTW, 2]


def kernel(query_embeddings, memory_keys, memory_values, top_k):
    import ml_dtypes

    assert int(top_k) == TOPK
    q = np.ascontiguousarray(np.asarray(query_embeddings, dtype=np.float32))
    k = np.ascontiguousarray(np.asarray(memory_keys, dtype=np.float32))
    v = np.ascontiguousarray(np.asarray(memory_values, dtype=np.float32))
    assert q.shape == (B, D) and k.shape == (N, D) and v.shape == (N, D)

    # host prep: fp64 normalize, bf16 cast, transpose, shard, pad
    kn = k.astype(np.float64)
    kn /= np.maximum(np.linalg.norm(kn, axis=1, keepdims=True), 1e-12)
    qn = q.astype(np.float64)
    qn /= np.maximum(np.linalg.norm(qn, axis=1, keepdims=True), 1e-12)

    qT = np.ascontiguousarray(qn.T).astype(ml_dtypes.bfloat16)  # [128, 1024]
    in_maps = []
    for c in range(NCORES):
        # keys as fp8 e4m3, scaled x64 so all entries sit in the normal
        # range (the screen is scale-invariant; host rescore is exact fp64)
        kTc = np.zeros((P, NPAD), dtype=ml_dtypes.float8_e4m3fn)
        kTc[:, :NLOC] = (
            np.ascontiguousarray(kn[c * NLOC : (c + 1) * NLOC].T) * 64.0
        ).astype(ml_dtypes.float8_e4m3fn)
        in_maps.append({"kT": kTc, "qT": qT})

    nc = _get_nc("full")
    if TRACE:
        _install_trace_shim()
    res = run_bass_kernel_spmd(
        nc, in_maps, core_ids=list(range(NCORES)), trace=TRACE
    )
    global LAST_EXEC_NS, LAST_RESULTS
    LAST_EXEC_NS = res.exec_time_ns
    LAST_RESULTS = res

    # host: top-NSEL pairs per (query, core) -> candidate members
    CPG = 2 * NSEL  # candidate rows per (query, core)
    mem_tab = _member_table()  # [OUTW, 2]
    cand = np.empty((B, NCORES * CPG), dtype=np.int64)
    for c in range(NCORES):
        pmf = np.asarray(res.results[c]["pm"]).astype(np.float32)  # [B, OUTW]
        part = np.argpartition(pmf, OUTW - NSEL, axis=1)[:, OUTW - NSEL :]
        mem = mem_tab[part].reshape(B, CPG)  # shard-local member rows
        gmem = mem + c * NLOC
        gmem[(mem >= NLOC) | (mem < 0)] = -1  # padded tail / no-member slots
        cand[:, c * CPG : (c + 1) * CPG] = gmem

    # exact fp64 rescore of candidates; invalid slots get -2 (< min cosine)
    z = np.full(cand.shape, -2.0, dtype=np.float64)
    step = 64
    for b0 in range(0, B, step):
        cb = cand[b0 : b0 + step]
        valid = cb >= 0
        kc = kn[np.clip(cb, 0, N - 1)]  # [step, C, D]
        zb = np.einsum("qcd,qd->qc", kc, qn[b0 : b0 + step])
        zb[~valid] = -2.0
        z[b0 : b0 + step] = zb

    # reference tie-break: larger sim first, then smaller index (stable top_k)
    order = np.lexsort((cand, -z), axis=1)[:, :TOPK]
    top_idx = np.take_along_axis(cand, order, axis=1)
    out = v[np.clip(top_idx, 0, N - 1)]
    return np.ascontiguousarray(out)


# revision 35
# speedup vs baseline: 1.0043x; 1.0043x over previous
"""Distributed exact cosine top-k retrieval (MemoryBank) on 8 trn2 NeuronCores.

Strategy (v4: split-PSUM cross-chunk fused drain+fold, fp8 keys):
  - memory_keys sharded row-wise across 8 cores; queries replicated.
  - Host prep: L2-normalize keys/queries in fp64; queries bf16, keys fp8
    e4m3 scaled x64 (all entries in the normal range; the candidate screen
    is scale-invariant and the host rescore is exact, so fp8 only adds
    ~0.003 screen noise vs a 0.049 validated margin).  Pre-transposed to
    [D, n]; key shard is 8 MB/core and stays RESIDENT in SBUF.
  - Per core, per query block (8 x 128 queries), stream 31 chunks of 2048
    keys through PSUM.  Each chunk uses TWO separate PSUM tiles (pshi =
    keys [1024:2048], pslo = keys [0:1024], 2 banks each, double
    buffered = all 8 banks).  The split matters: the Tile tracker
    serializes two engine readers of the SAME PSUM tile, which would kill
    ACT/DVE overlap.
  - Drain (the wall): sims leave PSUM as fp32 (TRN2 matmul cannot write
    16-bit PSUM) at 1 elem/cyc per engine port.  Per chunk, concurrently:
      ACT: copy pshi(c) -> SBUF bf16 "schi" (~1.11us saturated cadence)
      DVE: tensor_tensor MAX(pslo(c), schi(c-1)) -> stage bf16 (~1.13us)
    i.e. the pair-max fold is FUSED into the DVE's drain pass, pairing
    across adjacent chunks so ACT(c) and DVE(c) have no intra-chunk
    dependency.  Steady state is DVE-paced at ~1134ns/chunk with both
    drain engines >95% busy -- the PSUM-read duopoly floor.  (Engine
    rebalancing was measured: shifting whole chunks to ACT-only W=1
    shipping is a net loss; ACT saturates at ~1.12us/copy.)
  - PE: 4 bf16xfp8 matmuls (N=512, one PSUM bank each) per chunk at
    ~215ns issue gap; queries first in the DMA queue so compute starts at
    ~11us, not after the full key load.
  - Output: per (query block, core) 32 slots x 1024 bf16 pair-maxes
    (slot 0 pairs with a -3 junk partner; slot 31 = ACT's copy of the
    last chunk's hi half, written straight into the stage tile), shipped
    in 1 MiB staged DMAs, 65 MB/core total, overlapped.  qb0 is DMA-bound
    (key load + outputs share ~390 GB/s), so the first output stages are
    held back via tile_wait_until until the key load drains, with stage
    bufs=6 absorbing the staged folds in SBUF meanwhile.
  - Host: for each (query, core) take the top-NSEL pair-max slots, expand
    to member key ids, rescore candidates exactly in fp64 (normalized dot
    = cosine), global top-8 with the reference tie-break (desc sim, asc
    index), assemble output rows from memory_values.

Pair membership: slot (c, j), c in 0..30: members {c*2048+j} for c=0 else
{c*2048+j, (c-1)*2048+1024+j}; slot (31, j): {30*2048+1024+j}.

Selection depth NSEL=32 pairs/core, validated offline on the real dataset
(validate_screen.py): zero misses, worst margin 0.0488 for bf16 keys and
3.125/64 = 0.0488 for fp8 keys -- ~15 sigma above the sim noise.

History (HW-measured, rel err 0.0 throughout):
  1511555 ns  naive predecessor
   472542 ns  staged baseline (fp32 PSUM drained by match_replace/ACT at
              1x, fold as a separate DVE pass; DVE 85% busy on overhead)
   347290 ns  split-PSUM tiles + fused cross-chunk drain+fold
   343868 ns  + queries loaded before keys
   307917 ns  + fp8 keys, ACT-written edge slot, stage bufs 4, QSTEP 4
   303252 ns  + qb0 output-DMA deferral (tile_wait_until), stage bufs 6,
              split final stage DMA
"""

import numpy as np

import concourse.bacc as bacc
import concourse.bass as bass
import concourse.mybir as mybir
from concourse import tile
from concourse.bass_utils import run_bass_kernel_spmd

# problem sizes (hardcoded per contract)
B = 1024
N = 500000
D = 128
TOPK = 8
NCORES = 8
NLOC = N // NCORES  # 62500
CHUNK = 2048
NCHUNKS = (NLOC + CHUNK - 1) // CHUNK  # 31
NPAD = NCHUNKS * CHUNK  # 63488
P = 128
NQB = B // P  # 8 query blocks
GPC = CHUNK // 2  # 1024 pair-maxes per output slot
NSLOT = NCHUNKS + 1  # 31 chunk slots + 1 edge slot (last chunk's hi half)
OUTW = NSLOT * GPC  # 32768 pair-maxes per (query, core)

# host selection depth: top-NSEL pairs per (query, core) -> 2*NSEL candidate
# rows rescored exactly.
NSEL = 32

QSTEP = 4  # slots per staged out-DMA (4 * 1024 * 2B * 128 = 1 MiB)

_dt = mybir.dt


def build_kernel():
    """Build the per-core Bass program (SPMD: same program, different data)."""
    nc = bacc.Bacc(None, target_bir_lowering=False, debug=False)
    dt = _dt

    kT = nc.dram_tensor("kT", [P, NPAD], dt.float8e4, kind="ExternalInput")
    qT = nc.dram_tensor("qT", [P, B], dt.bfloat16, kind="ExternalInput")
    pm = nc.dram_tensor("pm", [B, OUTW], dt.bfloat16, kind="ExternalOutput")

    with tile.TileContext(nc) as tc:
        with (
            tc.tile_pool(name="kres", bufs=1) as kres,
            tc.tile_pool(name="qpool", bufs=1) as qpool,
            tc.tile_pool(name="scr", bufs=4) as scr,
            tc.tile_pool(name="stage", bufs=6) as stage,
            tc.tile_pool(name="psum", bufs=2, space="PSUM") as psum,
        ):
            # queries first (tiny, needed by every matmul; qb0's slice on its
            # own DMA so the first matmul can start earliest), then the
            # resident key shard chunk-by-chunk so qb0 compute starts as soon
            # as chunk 0 lands
            qt = qpool.tile([P, B], dt.bfloat16)
            nc.sync.dma_start(qt[:, 0:P], qT.ap()[:, 0:P])
            nc.sync.dma_start(qt[:, P:B], qT.ap()[:, P:B])
            kt = kres.tile([P, NPAD], dt.float8e4)
            for ch in range(NCHUNKS):
                nc.sync.dma_start(
                    kt[:, ch * CHUNK : (ch + 1) * CHUNK],
                    kT.ap()[:, ch * CHUNK : (ch + 1) * CHUNK],
                )
            # pair partner for slot 0 (never wins: all sims > -3)
            junk = qpool.tile([P, GPC], dt.bfloat16)
            nc.vector.memset(junk[:], -3.0)

            for qb in range(NQB):
                schi_prev = junk
                for s0 in range(0, NSLOT, QSTEP):
                    nslot = min(QSTEP, NSLOT - s0)
                    st = stage.tile([P, QSTEP * GPC], dt.bfloat16, tag="st")
                    for si in range(nslot):
                        s = s0 + si
                        if s < NCHUNKS:
                            c = s
                            # split lo/hi PSUM tiles so ACT (reads pshi) and
                            # DVE (reads pslo) never touch the same PSUM tile
                            # -- the Tile tracker serializes same-tile readers
                            pshi = psum.tile([P, GPC], dt.float32, tag="pshi")
                            pslo = psum.tile([P, GPC], dt.float32, tag="pslo")
                            for j in range(2):
                                nc.tensor.matmul(
                                    out=pshi[:, j * 512 : (j + 1) * 512],
                                    lhsT=qt[:, qb * P : (qb + 1) * P],
                                    rhs=kt[
                                        :,
                                        c * CHUNK + GPC + j * 512 : c * CHUNK
                                        + GPC
                                        + (j + 1) * 512,
                                    ],
                                    start=True,
                                    stop=True,
                                )
                            for j in range(2):
                                nc.tensor.matmul(
                                    out=pslo[:, j * 512 : (j + 1) * 512],
                                    lhsT=qt[:, qb * P : (qb + 1) * P],
                                    rhs=kt[
                                        :,
                                        c * CHUNK + j * 512 : c * CHUNK + (j + 1) * 512,
                                    ],
                                    start=True,
                                    stop=True,
                                )
                            # ACT: drain hi half -> SBUF bf16 (next chunk's
                            # pair partner).  For the last chunk the copy
                            # goes straight into the edge stage slot.
                            if c < NCHUNKS - 1:
                                schi_t = scr.tile(
                                    [P, GPC], dt.bfloat16, tag="sc", name="schi"
                                )
                                schi = schi_t[:]
                            else:
                                schi = st[:, (si + 1) * GPC : (si + 2) * GPC]
                            nc.scalar.copy(schi, pshi[:])
                            # DVE: fused drain+fold of lo half against the
                            # previous chunk's hi half
                            nc.vector.tensor_tensor(
                                out=st[:, si * GPC : (si + 1) * GPC],
                                in0=pslo[:],
                                in1=schi_prev,
                                op=mybir.AluOpType.max,
                            )
                            schi_prev = schi
                    if qb == 0 and s0 <= 12:
                        # qb0 is DMA-bound (key load + output stages share
                        # ~390 GB/s); hold the first output stages back so
                        # the resident-key load finishes unimpeded
                        with tc.tile_wait_until(0.028 + s0 * 0.0004):
                            nc.sync.dma_start(
                                pm.ap()[
                                    qb * P : (qb + 1) * P,
                                    s0 * GPC : (s0 + nslot) * GPC,
                                ],
                                st[:, : nslot * GPC],
                            )
                    elif qb == NQB - 1 and s0 == NSLOT - QSTEP:
                        # final stage: ship in two halves so the kernel tail
                        # only waits on a 0.5 MiB transfer
                        half = nslot // 2
                        nc.sync.dma_start(
                            pm.ap()[
                                qb * P : (qb + 1) * P,
                                s0 * GPC : (s0 + half) * GPC,
                            ],
                            st[:, : half * GPC],
                        )
                        nc.sync.dma_start(
                            pm.ap()[
                                qb * P : (qb + 1) * P,
                                (s0 + half) * GPC : (s0 + nslot) * GPC,
                            ],
                            st[:, half * GPC : nslot * GPC],
                        )
                    else:
                        nc.sync.dma_start(
                            pm.ap()[
                                qb * P : (qb + 1) * P,
                                s0 * GPC : (s0 + nslot) * GPC,
                            ],
                            st[:, : nslot * GPC],
                        )

    nc.compile()
    return nc


_NC_CACHE = {}

# test-harness knobs (the grading harness leaves these at defaults)
TRACE = False
LAST_EXEC_NS = None
LAST_RESULTS = None


def _get_nc(key):
    if key not in _NC_CACHE:
        _NC_CACHE[key] = build_kernel()
    return _NC_CACHE[key]


def _install_trace_shim():
    """Register the missing antenv.axon_hooks NTFF profile hook (dev only)."""
    import sys
    import types

    if "antenv.axon_hooks" in sys.modules:
        return
    from trn_agent_boot.trn_boot import _ntff_profile_via_ctypes

    hooks = types.ModuleType("antenv.axon_hooks")
    impl = _ntff_profile_via_ctypes("/opt/axon/libaxon_pjrt.so")
    hooks.get_axon_ntff_profile_hook = lambda: impl
    hooks.set_axon_ntff_profile_hook = lambda h: None
    sys.modules["antenv.axon_hooks"] = hooks

    import concourse.bass_utils as bu

    bu.upload_artifacts = lambda tmpdir: f"local:{tmpdir}"


def _member_table():
    """Map shipped column t in [0, OUTW) to its <=2 member key rows
    (shard-local, may exceed NLOC for zero-padded tail; -1 = no member)."""
    t_all = np.arange(OUTW, dtype=np.int64)
    s_all, j_all = np.divmod(t_all, GPC)
    m0 = np.where(s_all < NCHUNKS, s_all * CHUNK + j_all, (NCHUNKS - 1) * CHUNK + GPC + j_all)
    m1 = np.where(
        (s_all >= 1) & (s_all < NCHUNKS), (s_all - 1) * CHUNK + GPC + j_all, -1
    )
    return np.stack([m0, m1], axis=1)  # [OUTW, 2]


def kernel(query_embeddings, memory_keys, memory_values, top_k):
    import ml_dtypes

    assert int(top_k) == TOPK
    q = np.ascontiguousarray(np.asarray(query_embeddings, dtype=np.float32))
    k = np.ascontiguousarray(np.asarray(memory_keys, dtype=np.float32))
    v = np.ascontiguousarray(np.asarray(memory_values, dtype=np.float32))
    assert q.shape == (B, D) and k.shape == (N, D) and v.shape == (N, D)

    # host prep: fp64 normalize, bf16 cast, transpose, shard, pad
    kn = k.astype(np.float64)
    kn /= np.maximum(np.linalg.norm(kn, axis=1, keepdims=True), 1e-12)
    qn = q.astype(np.float64)
    qn /= np.maximum(np.linalg.norm(qn, axis=1, keepdims=True), 1e-12)

    qT = np.ascontiguousarray(qn.T).astype(ml_dtypes.bfloat16)  # [128, 1024]
    in_maps = []
    for c in range(NCORES):
        # keys as fp8 e4m3, scaled x64 so all entries sit in the normal
        # range (the screen is scale-invariant; host rescore is exact fp64)
        kTc = np.zeros((P, NPAD), dtype=ml_dtypes.float8_e4m3fn)
        kTc[:, :NLOC] = (
            np.ascontiguousarray(kn[c * NLOC : (c + 1) * NLOC].T) * 64.0
        ).astype(ml_dtypes.float8_e4m3fn)
        in_maps.append({"kT": kTc, "qT": qT})

    nc = _get_nc("full")
    if TRACE:
        _install_trace_shim()
    res = run_bass_kernel_spmd(
        nc, in_maps, core_ids=list(range(NCORES)), trace=TRACE
    )
    global LAST_EXEC_NS, LAST_RESULTS
    LAST_EXEC_NS = res.exec_time_ns
    LAST_RESULTS = res

    # host: top-NSEL pairs per (query, core) -> candidate members
    CPG = 2 * NSEL  # candidate rows per (query, core)
    mem_tab = _member_table()  # [OUTW, 2]
    cand = np.empty((B, NCORES * CPG), dtype=np.int64)
    for c in range(NCORES):
        pmf = np.asarray(res.results[c]["pm"]).astype(np.float32)  # [B, OUTW]
        part = np.argpartition(pmf, OUTW - NSEL, axis=1)[:, OUTW - NSEL :]
        mem = mem_tab[part].reshape(B, CPG)  # shard-local member rows
        gmem = mem + c * NLOC
        gmem[(mem >= NLOC) | (mem < 0)] = -1  # padded tail / no-member slots
        cand[:, c * CPG : (c + 1) * CPG] = gmem

    # exact fp64 rescore of candidates; invalid slots get -2 (< min cosine)
    z = np.full(cand.shape, -2.0, dtype=np.float64)
    step = 64
    for b0 in range(0, B, step):
        cb = cand[b0 : b0 + step]
        valid = cb >= 0
        kc = kn[np.clip(cb, 0, N - 1)]  # [step, C, D]
        zb = np.einsum("qcd,qd->qc", kc, qn[b0 : b0 + step])
        zb[~valid] = -2.0
        z[b0 : b0 + step] = zb

    # reference tie-break: larger sim first, then smaller index (stable top_k)
    order = np.lexsort((cand, -z), axis=1)[:, :TOPK]
    top_idx = np.take_along_axis(cand, order, axis=1)
    out = v[np.clip(top_idx, 0, N - 1)]
    return np.ascontiguousarray(out)
